# revision 9
# baseline (speedup 1.0000x reference)
"""Trainium2 Bass kernel for multi-head cross-attention block (nn_MCA).

Math (per batch b):
  q  = Wq  @ xq[b]   (1x1 conv)      k,v = Wkv @ x[b]
  per head h (32 heads, dh=8): attn = softmax(q_h^T k_h / sqrt(8))
  out = Wproj @ concat_h(attn @ v_h) + bias

Sharding: 8 cores = (batch b in 0..4) x (head-half in 0..2); each core handles
16 heads of one batch and produces a partial [256,1024] projection output;
host sums the two halves per batch and adds bias.

Device layout tricks:
  - scores^T computed as [k_tok, q_tok] tiles with K=dh=8 contraction, 4 heads
    packed concurrently in the PE array via 32-row tile_position groups.
  - heads live at 32-aligned partition offsets in scattered qT/kT tiles
    (partition 32g+d of tile j  <->  local head 4j+g, dim d).
  - exp on ScalarE reads 4 psum banks [128,2048] at once (amortizes the
    352-cycle ACT instruction overhead); scale 1/sqrt(8) folded into ACT.
  - attn@v computed transposed with a ones-augmented V (M=9), giving the
    softmax denominator for free; 4 heads packed via 32-col tile_position.
  - normalization (1/sum) applied once at the end on [128,1024] via a
    partition-broadcast DMA; projection partial stays on-core.
"""
import numpy as np

B, C = 4, 256
HEADS, DH = 32, 8
N = 1024                    # tokens (32*32), both for q and kv
SCALE = DH ** -0.5
NCORES = 8
NKT = 8                     # k tiles of 128 tokens
NQH = 2                     # q halves of 512 tokens
NJ = 4                      # rounds of 4 heads

_cache = {}


def _build():
    if "nc" in _cache:
        return _cache["nc"]
    import concourse.mybir as mybir
    import concourse.tile as tile
    from concourse import bacc

    F32 = mybir.dt.float32
    EXP = mybir.ActivationFunctionType.Exp

    from concourse.tile_rust import add_dep_helper

    nc = bacc.Bacc("TRN2", target_bir_lowering=False, debug=False,
                   num_devices=NCORES)

    # PE tiling-mode fence: the PE array must drain between tiling-mode
    # changes (32x128 scores vs 128x32 attn@v vs 128x128 proj), and Tile's
    # scheduler would otherwise interleave independent matmuls across the
    # boundary. A drain instruction + explicit dep edges pins the order.
    fence_state = {"prev": [], "drain": None}

    def mm(*args, **kwargs):
        inst = nc.tensor.matmul(*args, **kwargs)
        if fence_state["drain"] is not None:
            add_dep_helper(inst.ins, fence_state["drain"].ins, sync=False,
                           reason="pe mode-switch fence (after)")
        fence_state["prev"].append(inst)
        return inst

    def pe_fence():
        if not fence_state["prev"]:
            return
        dr = nc.tensor.drain()
        for p in fence_state["prev"]:
            add_dep_helper(dr.ins, p.ins, sync=False,
                           reason="pe mode-switch fence (before)")
        fence_state["prev"] = []
        fence_state["drain"] = dr

    xq_d = nc.dram_tensor("xq", [C, N], F32, kind="ExternalInput")
    x_d = nc.dram_tensor("x", [C, N], F32, kind="ExternalInput")
    wq_d = nc.dram_tensor("wq", [C, 512], F32, kind="ExternalInput")   # scattered cols
    wk_d = nc.dram_tensor("wk", [C, 512], F32, kind="ExternalInput")   # scattered cols
    wv_d = nc.dram_tensor("wv", [C, 128], F32, kind="ExternalInput")   # plain cols
    wp_d = nc.dram_tensor("wp", [128, C], F32, kind="ExternalInput")
    out_d = nc.dram_tensor("out", [C, N], F32, kind="ExternalOutput")
    dbg = {}
    if _cache.get("debug"):
        for nm, shp in [("qT_o", [128, 4096]), ("kT_o", [128, 4096]),
                        ("v9_o", [128, NKT * 144]), ("cat_o", [128, N]),
                        ("s_o", [16, N]), ("e_o", [128, 2048]),
                        ("rb_o", [128, N])]:
            dbg[nm] = nc.dram_tensor(nm, shp, F32, kind="ExternalOutput")

    with tile.TileContext(nc) as tc:
        from contextlib import ExitStack
        with ExitStack() as st:
            pp = st.enter_context(tc.tile_pool(name="persist", bufs=1))
            xq_sb = pp.tile([128, 2048], F32, name="xq_sb")   # chunk c at c*1024
            x_sb = pp.tile([128, 2048], F32, name="x_sb")
            wq_sb = pp.tile([128, 1024], F32, name="wq_sb")   # chunk c at c*512
            wk_sb = pp.tile([128, 1024], F32, name="wk_sb")
            wv_sb = pp.tile([128, 256], F32, name="wv_sb")    # chunk c at c*128
            wp_sb = pp.tile([128, 256], F32, name="wp_sb")
            qT = pp.tile([128, 4096], F32, name="qT")         # tile j at j*1024
            kT = pp.tile([128, 4096], F32, name="kT")
            v9 = pp.tile([128, NKT * 144], F32, name="v9")    # [ktok, kt*144 + h*9 + d]
            attn_cat = pp.tile([128, N], F32, name="attn_cat")
            s_cat = pp.tile([16, N], F32, name="s_cat")
            r_cat = pp.tile([16, N], F32, name="r_cat")
            rb = pp.tile([128, N], F32, name="rb")
            attn_n = pp.tile([128, N], F32, name="attn_n")

            for c in range(2):
                nc.sync.dma_start(out=xq_sb[:, c * 1024:(c + 1) * 1024],
                                  in_=xq_d.ap()[c * 128:(c + 1) * 128, :])
                nc.sync.dma_start(out=x_sb[:, c * 1024:(c + 1) * 1024],
                                  in_=x_d.ap()[c * 128:(c + 1) * 128, :])
                nc.sync.dma_start(out=wq_sb[:, c * 512:(c + 1) * 512],
                                  in_=wq_d.ap()[c * 128:(c + 1) * 128, :])
                nc.sync.dma_start(out=wk_sb[:, c * 512:(c + 1) * 512],
                                  in_=wk_d.ap()[c * 128:(c + 1) * 128, :])
                nc.sync.dma_start(out=wv_sb[:, c * 128:(c + 1) * 128],
                                  in_=wv_d.ap()[c * 128:(c + 1) * 128, :])
            nc.sync.dma_start(out=wp_sb, in_=wp_d.ap())
            nc.vector.memset(v9, 1.0)

            # ---- phase A: projections ----
            with tc.tile_pool(name="psA", bufs=2, space="PSUM") as psA:
                # qT / kT scattered tiles: psum[32g+d, pix] for round j
                for name, w_sb, dst in (("q", wq_sb, qT), ("k", wk_sb, kT)):
                    for j in range(NJ):
                        for qh in range(NQH):
                            ps = psA.tile([128, 512], F32, name=f"ps{name}{j}{qh}",
                                          tag="qk")
                            for cc in range(2):
                                mm(
                                    out=ps,
                                    lhsT=w_sb[:, cc * 512 + 128 * j:
                                              cc * 512 + 128 * j + 128],
                                    rhs=xq_sb[:, cc * 1024 + qh * 512:
                                              cc * 1024 + (qh + 1) * 512]
                                    if name == "q" else
                                    x_sb[:, cc * 1024 + qh * 512:
                                         cc * 1024 + (qh + 1) * 512],
                                    start=(cc == 0), stop=(cc == 1),
                                )
                            nc.scalar.copy(
                                out=dst[:, j * 1024 + qh * 512:
                                        j * 1024 + (qh + 1) * 512],
                                in_=ps)
                # v in [token, channel] layout (one matmul per k-tile)
                for kt in range(NKT):
                    ps = psA.tile([128, 128], F32, name=f"psv{kt}", tag="v")
                    for cc in range(2):
                        mm(
                            out=ps,
                            lhsT=x_sb[:, cc * 1024 + kt * 128:
                                      cc * 1024 + (kt + 1) * 128],
                            rhs=wv_sb[:, cc * 128:(cc + 1) * 128],
                            start=(cc == 0), stop=(cc == 1),
                        )
                    nc.vector.tensor_copy(
                        out=v9[:, kt * 144:(kt + 1) * 144].rearrange(
                            "p (h d) -> p h d", d=9)[:, :, 0:8],
                        in_=ps.rearrange("p (h d) -> p h d", d=8),
                    )

            # ---- main loop: scores -> exp -> attn@v ----
            pe_fence()
            ep = st.enter_context(tc.tile_pool(name="epool", bufs=10))
            with tc.tile_pool(name="smm", bufs=2, space="PSUM") as sp:
                for qh in range(NQH):
                    for j in range(NJ):
                        e_tiles = []
                        for kt in range(NKT):
                            ps_s = sp.tile([128, 2048], F32,
                                           name=f"s{qh}{j}{kt}", tag="s")
                            for g in range(4):
                                mm(
                                    out=ps_s[:, g * 512:(g + 1) * 512],
                                    lhsT=kT[32 * g:32 * g + 8,
                                            j * 1024 + kt * 128:
                                            j * 1024 + (kt + 1) * 128],
                                    rhs=qT[32 * g:32 * g + 8,
                                           j * 1024 + qh * 512:
                                           j * 1024 + (qh + 1) * 512],
                                    start=True, stop=True,
                                    tile_position=(32 * g, 0),
                                )
                            e = ep.tile([128, 2048], F32,
                                        name=f"e{qh}{j}{kt}", tag="e")
                            nc.scalar.activation(out=e, in_=ps_s, func=EXP,
                                                 scale=SCALE)
                            if dbg and qh == 0 and j == 0 and kt == 0:
                                nc.sync.dma_start(out=dbg["e_o"].ap(), in_=e)
                            e_tiles.append(e)
                        pe_fence()
                        ps_o = sp.tile([128, 512], F32, name=f"o{qh}{j}",
                                       tag="s")
                        for kt in range(NKT):
                            for g in range(4):
                                mm(
                                    out=ps_o[32 * g:32 * g + 9, :],
                                    lhsT=v9[:, kt * 144 + (4 * j + g) * 9:
                                            kt * 144 + (4 * j + g) * 9 + 9],
                                    rhs=e_tiles[kt][:, g * 512:(g + 1) * 512],
                                    start=(kt == 0), stop=(kt == NKT - 1),
                                    tile_position=(0, 32 * g),
                                )
                        pe_fence()
                        # evacuate psum, then gather: attn rows + sum rows
                        o_st = ep.tile([128, 512], F32, name=f"ost{qh}{j}",
                                       tag="ost")
                        nc.vector.tensor_copy(o_st, ps_o)
                        # NOTE: only AP dim 0 crosses partitions -> one DMA
                        # per 32-row group.
                        for g in range(4):
                            nc.sync.dma_start(
                                out=attn_cat[32 * j + 8 * g:32 * j + 8 * g + 8,
                                             qh * 512:(qh + 1) * 512],
                                in_=o_st[32 * g:32 * g + 8, :],
                            )
                            nc.sync.dma_start(
                                out=s_cat[4 * j + g:4 * j + g + 1,
                                          qh * 512:(qh + 1) * 512],
                                in_=o_st[32 * g + 8:32 * g + 9, :],
                            )

            # ---- tail: normalize + projection ----
            nc.vector.reciprocal(r_cat, s_cat)
            nc.gpsimd.dma_start(out=rb,
                                in_=r_cat.unsqueeze(1).broadcast_to([16, 8, N]))
            nc.vector.tensor_mul(attn_n, attn_cat, rb)
            if dbg:
                nc.sync.dma_start(out=dbg["qT_o"].ap(), in_=qT)
                nc.sync.dma_start(out=dbg["kT_o"].ap(), in_=kT)
                nc.sync.dma_start(out=dbg["v9_o"].ap(), in_=v9)
                nc.sync.dma_start(out=dbg["cat_o"].ap(), in_=attn_cat)
                nc.sync.dma_start(out=dbg["s_o"].ap(), in_=s_cat)
                nc.sync.dma_start(out=dbg["rb_o"].ap(), in_=rb)
            out_sb = pp.tile([128, 2048], F32, name="out_sb")
            with tc.tile_pool(name="ptail", bufs=2, space="PSUM") as pt:
                for ot in range(2):
                    for qh in range(NQH):
                        ps_p = pt.tile([128, 512], F32, name=f"pp{ot}{qh}",
                                       tag="p")
                        mm(
                            out=ps_p,
                            lhsT=wp_sb[:, ot * 128:(ot + 1) * 128],
                            rhs=attn_n[:, qh * 512:(qh + 1) * 512],
                            start=True, stop=True,
                        )
                        nc.scalar.copy(
                            out=out_sb[:, ot * 1024 + qh * 512:
                                       ot * 1024 + (qh + 1) * 512],
                            in_=ps_p)
                for ot in range(2):
                    nc.sync.dma_start(
                        out=out_d.ap()[ot * 128:(ot + 1) * 128, :],
                        in_=out_sb[:, ot * 1024:(ot + 1) * 1024])

    nc.compile()
    _cache["nc"] = nc
    return nc


def _prep_core(core, xq, x, Wq, Wkv, Wproj):
    b, half = core // 2, core % 2
    xq_np = np.ascontiguousarray(xq[b].reshape(C, N))
    x_np = np.ascontiguousarray(x[b].reshape(C, N))

    # scattered column permutation: local head h=4j+g, dim d -> col 128j+32g+d
    hl = np.arange(16)
    d = np.arange(8)
    colperm = (128 * (hl[:, None] // 4) + 32 * (hl[:, None] % 4) + d[None, :]).reshape(-1)

    wq_block = Wq[128 * half:128 * half + 128, :]          # [128, 256] rows 8h+d
    wq_scat = np.zeros((C, 512), np.float32)
    wq_scat[:, colperm] = wq_block.T
    wk_block = Wkv[128 * half:128 * half + 128, :]
    wk_scat = np.zeros((C, 512), np.float32)
    wk_scat[:, colperm] = wk_block.T
    wv_rhs = np.ascontiguousarray(Wkv[256 + 128 * half:256 + 128 * half + 128, :].T)
    wp = np.ascontiguousarray(Wproj[:, 128 * half:128 * half + 128].T)
    return {"xq": xq_np, "x": x_np, "wq": wq_scat, "wk": wk_scat,
            "wv": wv_rhs, "wp": wp}


def run_internal(inputs, trace=False):
    from concourse.bass_utils import run_bass_kernel_spmd
    nc = _build()
    xq, x = inputs["xq"], inputs["x"]
    Wq, Wkv = np.asarray(inputs["Wq"]), np.asarray(inputs["Wkv"])
    Wproj, bproj = np.asarray(inputs["Wproj"]), np.asarray(inputs["bproj"])
    in_maps = [_prep_core(c, np.asarray(xq), np.asarray(x), Wq, Wkv, Wproj)
               for c in range(NCORES)]
    res = run_bass_kernel_spmd(nc, in_maps, list(range(NCORES)), trace=trace)
    out = np.zeros((B, C, 32, 32), np.float32)
    for b in range(B):
        part = res.results[2 * b]["out"] + res.results[2 * b + 1]["out"]
        out[b] = (part + bproj[:, None]).reshape(C, 32, 32)
    return out, res


def kernel(**inputs):
    out, _ = run_internal(inputs, trace=False)
    return out


# revision 14
# speedup vs baseline: 1.0512x; 1.0512x over previous
"""Trainium2 Bass kernel for multi-head cross-attention block (nn_MCA).

Math (per batch b):
  q  = Wq  @ xq[b]   (1x1 conv)      k,v = Wkv @ x[b]
  per head h (32 heads, dh=8): attn = softmax(q_h^T k_h / sqrt(8))
  out = Wproj @ concat_h(attn @ v_h) + bias

Sharding: 8 cores = (batch b in 0..4) x (head-half in 0..2); each core handles
16 heads of one batch and produces a partial [256,1024] projection output;
host sums the two halves per batch and adds bias.

Device layout:
  - scores^T computed as [k_tok, q_tok] psum tiles with K=dh=8 contraction;
    4 heads run CONCURRENTLY in the PE array via 32-row tile_position groups
    (heads live at 32-aligned partition offsets of scattered qT/kT tiles:
    partition 32g+d of tile j <-> local head 4j+g, dim d).
  - exp on ScalarE reads 4 psum banks [128,2048] at once (amortizes ACT
    instruction overhead); the 1/sqrt(8) scale is folded into the ACT affine.
    ScalarE is the bottleneck engine (~16.8M exp elements per core); the
    whole schedule exists to keep it 100% busy.
  - attn@v computed transposed with a ones-augmented V (M=9 stationary),
    giving the softmax denominator for free; 4 heads packed via 32-col
    tile_position into one psum bank.
  - all psum usage shares one 2-slot x 4-bank pool so j=1..3 q/k/v
    projections can be deferred into the first exp stream (short startup).
  - normalization (1/sum) applied once at the end on [128,1024] via a
    partition-broadcast DMA + one multiply; projection partial stays on-core.
"""
import numpy as np

B, C = 4, 256
HEADS, DH = 32, 8
N = 1024                    # tokens (32*32), both for q and kv
SCALE = DH ** -0.5
NCORES = 8
NKT = 8                     # k tiles of 128 tokens
NQH = 2                     # q halves of 512 tokens
NJ = 4                      # rounds of 4 heads

_cache = {}


def _build():
    if "nc" in _cache:
        return _cache["nc"]
    import concourse.mybir as mybir
    import concourse.tile as tile
    from concourse import bacc

    F32 = mybir.dt.float32
    EXP = mybir.ActivationFunctionType.Exp

    nc = bacc.Bacc("TRN2", target_bir_lowering=False, debug=False,
                   num_devices=NCORES)
    mm = nc.tensor.matmul

    xq_d = nc.dram_tensor("xq", [C, N], F32, kind="ExternalInput")
    x_d = nc.dram_tensor("x", [C, N], F32, kind="ExternalInput")
    wq_d = nc.dram_tensor("wq", [C, 512], F32, kind="ExternalInput")   # scattered cols
    wk_d = nc.dram_tensor("wk", [C, 512], F32, kind="ExternalInput")   # scattered cols
    wv_d = nc.dram_tensor("wv", [C, 128], F32, kind="ExternalInput")   # plain cols
    wp_d = nc.dram_tensor("wp", [128, C], F32, kind="ExternalInput")
    out_d = nc.dram_tensor("out", [C, N], F32, kind="ExternalOutput")
    dbg = {}
    if _cache.get("debug"):
        for nm, shp in [("qT_o", [128, 4096]), ("kT_o", [128, 4096]),
                        ("v9_o", [128, NKT * 144]), ("cat_o", [128, N]),
                        ("s_o", [16, N]), ("e_o", [128, 2048]),
                        ("rb_o", [128, N])]:
            dbg[nm] = nc.dram_tensor(nm, shp, F32, kind="ExternalOutput")

    REP = _cache.get("repeat", 1)
    interleave = REP == 1

    with tile.TileContext(nc) as tc:
        from contextlib import ExitStack
        with ExitStack() as st:
            pp = st.enter_context(tc.tile_pool(name="persist", bufs=1))
            xq_sb = pp.tile([128, 2048], F32, name="xq_sb")   # chunk c at c*1024
            x_sb = pp.tile([128, 2048], F32, name="x_sb")
            wq_sb = pp.tile([128, 1024], F32, name="wq_sb")   # chunk c at c*512
            wk_sb = pp.tile([128, 1024], F32, name="wk_sb")
            wv_sb = pp.tile([128, 256], F32, name="wv_sb")    # chunk c at c*128
            wp_sb = pp.tile([128, 256], F32, name="wp_sb")
            qT = pp.tile([128, 4096], F32, name="qT")         # tile j at j*1024
            kT = pp.tile([128, 4096], F32, name="kT")
            v9 = pp.tile([128, NKT * 144], F32, name="v9")    # [ktok, kt*144 + h*9 + d]
            attn_cat = pp.tile([128, N], F32, name="attn_cat")
            s_cat = pp.tile([16, N], F32, name="s_cat")
            r_cat = pp.tile([16, N], F32, name="r_cat")
            rb = pp.tile([128, N], F32, name="rb")
            attn_n = pp.tile([128, N], F32, name="attn_n")

            # --- input DMAs: what the j=0 projections need goes first ---
            for c in range(2):
                nc.sync.dma_start(out=xq_sb[:, c * 1024:(c + 1) * 1024],
                                  in_=xq_d.ap()[c * 128:(c + 1) * 128, :])
                nc.sync.dma_start(out=x_sb[:, c * 1024:(c + 1) * 1024],
                                  in_=x_d.ap()[c * 128:(c + 1) * 128, :])
                nc.sync.dma_start(out=wq_sb[:, c * 512:c * 512 + 128],
                                  in_=wq_d.ap()[c * 128:(c + 1) * 128, 0:128])
                nc.sync.dma_start(out=wk_sb[:, c * 512:c * 512 + 128],
                                  in_=wk_d.ap()[c * 128:(c + 1) * 128, 0:128])
            for c in range(2):
                nc.sync.dma_start(out=wq_sb[:, c * 512 + 128:(c + 1) * 512],
                                  in_=wq_d.ap()[c * 128:(c + 1) * 128, 128:512])
                nc.sync.dma_start(out=wk_sb[:, c * 512 + 128:(c + 1) * 512],
                                  in_=wk_d.ap()[c * 128:(c + 1) * 128, 128:512])
                nc.sync.dma_start(out=wv_sb[:, c * 128:(c + 1) * 128],
                                  in_=wv_d.ap()[c * 128:(c + 1) * 128, :])
            nc.sync.dma_start(out=wp_sb, in_=wp_d.ap())
            nc.vector.memset(v9, 1.0)

            # one shared psum pool: 2 slots x 4 banks
            sp = st.enter_context(tc.tile_pool(name="smm", bufs=2, space="PSUM"))
            ep = st.enter_context(
                tc.tile_pool(name="epool", bufs=_cache.get("ebufs", 10)))

            def proj_qk(j):
                for name, w_sb, src, dst in (("q", wq_sb, xq_sb, qT),
                                             ("k", wk_sb, x_sb, kT)):
                    for qh in range(NQH):
                        ps = sp.tile([128, 512], F32,
                                     name=f"ps{name}{j}{qh}", tag="s")
                        for cc in range(2):
                            mm(out=ps,
                               lhsT=w_sb[:, cc * 512 + 128 * j:
                                         cc * 512 + 128 * j + 128],
                               rhs=src[:, cc * 1024 + qh * 512:
                                       cc * 1024 + (qh + 1) * 512],
                               start=(cc == 0), stop=(cc == 1))
                        nc.vector.tensor_copy(
                            dst[:, j * 1024 + qh * 512:
                                j * 1024 + (qh + 1) * 512], ps)

            def proj_v():
                for kt in range(NKT):
                    ps = sp.tile([128, 128], F32, name=f"psv{kt}", tag="s")
                    for cc in range(2):
                        mm(out=ps,
                           lhsT=x_sb[:, cc * 1024 + kt * 128:
                                     cc * 1024 + (kt + 1) * 128],
                           rhs=wv_sb[:, cc * 128:(cc + 1) * 128],
                           start=(cc == 0), stop=(cc == 1))
                    nc.vector.tensor_copy(
                        v9[:, kt * 144:(kt + 1) * 144].rearrange(
                            "p (h d) -> p h d", d=9)[:, :, 0:8],
                        ps.rearrange("p (h d) -> p h d", d=8))

            def scores_exp(rep, qh, j):
                e_tiles = []
                for kt in range(NKT):
                    ps_s = sp.tile([128, 2048], F32,
                                   name=f"s{rep}_{qh}{j}{kt}", tag="s")
                    for g in range(4):
                        mm(out=ps_s[:, g * 512:(g + 1) * 512],
                           lhsT=kT[32 * g:32 * g + 8,
                                   j * 1024 + kt * 128:
                                   j * 1024 + (kt + 1) * 128],
                           rhs=qT[32 * g:32 * g + 8,
                                  j * 1024 + qh * 512:
                                  j * 1024 + (qh + 1) * 512],
                           start=True, stop=True,
                           tile_position=(32 * g, 0))
                    e = ep.tile([128, 2048], F32,
                                name=f"e{rep}_{qh}{j}{kt}", tag="e")
                    nc.scalar.activation(out=e, in_=ps_s, func=EXP, scale=SCALE)
                    if dbg and rep == 0 and qh == 0 and j == 0 and kt == 0:
                        nc.sync.dma_start(out=dbg["e_o"].ap(), in_=e)
                    e_tiles.append(e)
                return e_tiles

            def attnv(rep, qh, j, e_tiles):
                ps_o = sp.tile([128, 512], F32, name=f"o{rep}_{qh}{j}", tag="s")
                for kt in range(NKT):
                    for g in range(4):
                        mm(out=ps_o[32 * g:32 * g + 9, :],
                           lhsT=v9[:, kt * 144 + (4 * j + g) * 9:
                                   kt * 144 + (4 * j + g) * 9 + 9],
                           rhs=e_tiles[kt][:, g * 512:(g + 1) * 512],
                           start=(kt == 0), stop=(kt == NKT - 1),
                           tile_position=(0, 32 * g))
                o_st = ep.tile([128, 512], F32, name=f"ost{rep}_{qh}{j}",
                               tag="ost")
                nc.vector.tensor_copy(o_st, ps_o)
                # only AP dim 0 crosses partitions -> one DMA per 32-row group
                for g in range(4):
                    nc.sync.dma_start(
                        out=attn_cat[32 * j + 8 * g:32 * j + 8 * g + 8,
                                     qh * 512:(qh + 1) * 512],
                        in_=o_st[32 * g:32 * g + 8, :])
                    nc.sync.dma_start(
                        out=s_cat[4 * j + g:4 * j + g + 1,
                                  qh * 512:(qh + 1) * 512],
                        in_=o_st[32 * g + 8:32 * g + 9, :])

            if interleave:
                # j=0 projections, then round (0,0) scores immediately; defer
                # the remaining projections into the first exp stream.
                proj_qk(0)
                e00 = scores_exp(0, 0, 0)
                for j in range(1, NJ):
                    proj_qk(j)
                proj_v()
                attnv(0, 0, 0, e00)
                rounds = [(qh, j) for qh in range(NQH) for j in range(NJ)][1:]
                for qh, j in rounds:
                    attnv(0, qh, j, scores_exp(0, qh, j))
            else:
                for j in range(NJ):
                    proj_qk(j)
                proj_v()
                with tc.For_i(0, REP):
                    for qh in range(NQH):
                        for j in range(NJ):
                            attnv(0, qh, j, scores_exp(0, qh, j))

            if dbg:
                nc.sync.dma_start(out=dbg["qT_o"].ap(), in_=qT)
                nc.sync.dma_start(out=dbg["kT_o"].ap(), in_=kT)
                nc.sync.dma_start(out=dbg["v9_o"].ap(), in_=v9)
                nc.sync.dma_start(out=dbg["cat_o"].ap(), in_=attn_cat)
                nc.sync.dma_start(out=dbg["s_o"].ap(), in_=s_cat)

            # ---- tail: normalize + projection ----
            nc.vector.reciprocal(r_cat, s_cat)
            nc.gpsimd.dma_start(out=rb,
                                in_=r_cat.unsqueeze(1).broadcast_to([16, 8, N]))
            if dbg:
                nc.sync.dma_start(out=dbg["rb_o"].ap(), in_=rb)
            nc.vector.tensor_mul(attn_n, attn_cat, rb)
            out_sb = pp.tile([128, 2048], F32, name="out_sb")
            for ot in range(2):
                for qh in range(NQH):
                    ps_p = sp.tile([128, 512], F32, name=f"pp{ot}{qh}", tag="s")
                    mm(out=ps_p,
                       lhsT=wp_sb[:, ot * 128:(ot + 1) * 128],
                       rhs=attn_n[:, qh * 512:(qh + 1) * 512],
                       start=True, stop=True)
                    nc.vector.tensor_copy(
                        out_sb[:, ot * 1024 + qh * 512:
                               ot * 1024 + (qh + 1) * 512], ps_p)
            for ot in range(2):
                nc.sync.dma_start(
                    out=out_d.ap()[ot * 128:(ot + 1) * 128, :],
                    in_=out_sb[:, ot * 1024:(ot + 1) * 1024])

    nc.compile()
    _cache["nc"] = nc
    return nc


def _prep_core(core, xq, x, Wq, Wkv, Wproj):
    half = core % 2
    b = core // 2
    xq_np = np.ascontiguousarray(xq[b].reshape(C, N))
    x_np = np.ascontiguousarray(x[b].reshape(C, N))

    # scattered column permutation: local head h=4j+g, dim d -> col 128j+32g+d
    hl = np.arange(16)
    d = np.arange(8)
    colperm = (128 * (hl[:, None] // 4) + 32 * (hl[:, None] % 4)
               + d[None, :]).reshape(-1)

    wq_block = Wq[128 * half:128 * half + 128, :]          # rows 8h+d
    wq_scat = np.zeros((C, 512), np.float32)
    wq_scat[:, colperm] = wq_block.T
    wk_block = Wkv[128 * half:128 * half + 128, :]
    wk_scat = np.zeros((C, 512), np.float32)
    wk_scat[:, colperm] = wk_block.T
    wv_rhs = np.ascontiguousarray(
        Wkv[256 + 128 * half:256 + 128 * half + 128, :].T)
    wp = np.ascontiguousarray(Wproj[:, 128 * half:128 * half + 128].T)
    return {"xq": xq_np, "x": x_np, "wq": wq_scat, "wk": wk_scat,
            "wv": wv_rhs, "wp": wp}


def run_internal(inputs, trace=False):
    from concourse.bass_utils import run_bass_kernel_spmd
    nc = _build()
    xq, x = np.asarray(inputs["xq"]), np.asarray(inputs["x"])
    Wq, Wkv = np.asarray(inputs["Wq"]), np.asarray(inputs["Wkv"])
    Wproj, bproj = np.asarray(inputs["Wproj"]), np.asarray(inputs["bproj"])
    in_maps = [_prep_core(c, xq, x, Wq, Wkv, Wproj) for c in range(NCORES)]
    res = run_bass_kernel_spmd(nc, in_maps, list(range(NCORES)), trace=trace)
    out = np.zeros((B, C, 32, 32), np.float32)
    for b in range(B):
        part = res.results[2 * b]["out"] + res.results[2 * b + 1]["out"]
        out[b] = (part + bproj[:, None]).reshape(C, 32, 32)
    return out, res


def kernel(**inputs):
    out, _ = run_internal(inputs, trace=False)
    return out
